# revision 17
# baseline (speedup 1.0000x reference)
"""Trainium2 Bass kernel for nn_DiscreteNormalization (WiSARD-style weightless NN).

Reference semantics:
    bits = x[conn]                    # [S, N, B] gather of binary x
    addr = sum_j bits[...,j] << j     # [S, N] 12-bit RAM addresses
    out  = memory[s, n, addr[s,n]]    # [S, N] RAM lookup
    votes= sum_s out                  # [N]
    y    = (votes > S/2).astype(f32)  # [N]

The neuron axis is sharded across the 8 cores (each core owns all 8 sub-nets
for its 1024 neurons -> no cross-core reduction). Partition p of a core owns
neurons n = p*8 + n1, n1 in [0,8).

The 2^B-cell RAM tables store single bits, so each neuron's whole 4096-cell
table bit-packs into 128 int32 words = 512 B (host-side in make_in_maps,
laid out [16 w_lo, 8 w_hi] per neuron). The per-iteration table read is then
ONE statically-addressed contiguous 4 MB dma_start per core instead of a
data-dependent dma_gather; the 12-bit address picks its word with a cheap
two-level one-hot select (8-way then 16-way) and the bit with a per-element
variable shift, all on DVE int ops.

x-gather runs via gpsimd.ap_gather from a [128, 8192] replicated f32 copy of
x. Indices are each partition's own conn row; the op's 16-partition
wrapped-index semantics replicate each group's gathered stream across its 16
partitions, so a fused (diag-mask * 2^j) multiply + reduce both selects each
partition's diagonal and packs the 12-bit address in one pass. On TRN2 the
Pool engine and DVE serialize on the shared SBUF port pair, so nothing
overlaps the gather; the kernel therefore minimizes total serialized work
(single-shot gather, two-level select) rather than chasing overlap.
"""

import numpy as np

import concourse.bacc as bacc
import concourse.bass as bass
import concourse.mybir as mybir
from concourse.bass_utils import run_bass_kernel_spmd
from concourse.tile import TileContext

S, N, B, IB = 8, 8192, 12, 8192
A = 1 << B                    # 4096 cells per neuron
NCORES = 8
NPC = N // NCORES             # 1024 neurons per core
P = 128
NPP = NPC // P                # 8 neurons per partition
SN = S * NPP                  # 64 (s, n1) pairs per partition
NW = A // 32                  # 128 packed int32 words per neuron table
WL, WH = 16, 8                # word index split: w = wh*16 + wl
I32 = mybir.dt.int32
I16 = mybir.dt.int16
F32 = mybir.dt.float32
ALU = mybir.AluOpType
AX = mybir.AxisListType

_cache: dict = {}


def build(loop_iters: int | None = None, unroll: int = 1,
          bf16: bool = False, empty: bool = False):
    nc = bacc.Bacc("TRN2", debug=False, enable_asserts=False,
                   num_devices=NCORES, enable_partition_id=False)
    x_d = nc.dram_tensor("x", [IB], I32, kind="ExternalInput")
    conn_d = nc.dram_tensor("conn", [S, NPC, B], I32, kind="ExternalInput")
    mem_d = nc.dram_tensor("mem", [P, SN * NW], I32, kind="ExternalInput")
    y_d = nc.dram_tensor("y", [NPC], F32, kind="ExternalOutput")
    xf_d = nc.dram_tensor("xf_scratch", [1, IB], F32, kind="Internal")

    conn_p = conn_d.ap().rearrange("s (p n1) j -> p s n1 j", p=P)
    y_p = y_d.ap().rearrange("(p n1) -> p n1", p=P)

    with TileContext(nc) as tc:
        with (tc.tile_pool(name="const", bufs=1) as cpool,
              tc.tile_pool(name="work", bufs=1) as pool):
            # ---- constants ------------------------------------------------
            # W[p, j*16+r] = (r == p%16) * 2^j   (f32, exact)
            w_r = cpool.tile([P, B, 16], I32)
            nc.gpsimd.iota(w_r[:], pattern=[[0, B], [1, 16]], channel_multiplier=0)
            w_pm = cpool.tile([P, 1], I32)
            nc.gpsimd.iota(w_pm[:], pattern=[[0, 1]], channel_multiplier=1)
            nc.vector.tensor_scalar(out=w_pm[:], in0=w_pm[:], scalar1=15,
                                    scalar2=None, op0=ALU.bitwise_and)
            w_i = cpool.tile([P, B, 16], I32)
            nc.vector.tensor_tensor(out=w_i[:], in0=w_r[:],
                                    in1=w_pm[:].to_broadcast([P, B, 16]),
                                    op=ALU.is_equal)
            w_j2 = cpool.tile([P, B, 16], I32)
            nc.gpsimd.iota(w_j2[:], pattern=[[1, B], [0, 16]], channel_multiplier=0)
            nc.vector.tensor_tensor(out=w_i[:], in0=w_i[:], in1=w_j2[:],
                                    op=ALU.logical_shift_left)  # onehot << j
            W = cpool.tile([P, B * 16], F32)
            nc.vector.tensor_copy(out=W[:], in_=w_i[:].rearrange("p a b -> p (a b)"))
            W16 = cpool.tile([P, B * 16], mybir.dt.bfloat16)
            nc.vector.tensor_copy(out=W16[:], in_=w_i[:].rearrange("p a b -> p (a b)"))

            # iota rows for the two-level word select
            Giota = cpool.tile([P, WH], I32)
            nc.gpsimd.iota(Giota[:], pattern=[[1, WH]], channel_multiplier=0)
            Liota = cpool.tile([P, WL], I32)
            nc.gpsimd.iota(Liota[:], pattern=[[1, WL]], channel_multiplier=0)

            # x -> f32 -> DRAM scratch -> broadcast to all 128 partitions
            x_row = cpool.tile([16, IB // 16], I32)
            nc.sync.dma_start(out=x_row[:],
                              in_=x_d.ap().rearrange("(a b) -> a b", a=16))
            xf_row = cpool.tile([16, IB // 16], F32)
            nc.vector.tensor_copy(out=xf_row[:], in_=x_row[:])
            nc.sync.dma_start(out=xf_d.ap().rearrange("o (a b) -> (o a) b", a=16),
                              in_=xf_row[:])
            XT = cpool.tile([P, IB], F32)
            nc.sync.dma_start(out=XT[:], in_=xf_d.ap().to_broadcast([P, IB]))

            # conn -> int16 indices
            CT = cpool.tile([P, SN, B], I32)
            nc.sync.dma_start(out=CT[:], in_=conn_p)
            CT16 = cpool.tile([P, SN * B], I16)
            nc.vector.tensor_copy(out=CT16[:], in_=CT[:].rearrange("p a b -> p (a b)"))

            Rt = cpool.tile([P, SN, WL, WH], I32)  # packed tables (32 KB/part)

            def body(_=None):
                if empty:
                    res = pool.tile([P, NPP], F32, tag="res")
                    nc.vector.memset(res[:], 0.0)
                    nc.sync.dma_start(out=y_p, in_=res[:])
                    return
                # whole packed table for this core: 4 MB contiguous stream
                nc.sync.dma_start(
                    out=Rt[:],
                    in_=mem_d.ap().rearrange("p (a b c) -> p a b c", b=WL, c=WH))

                # ---- x-bit gather + 12-bit address pack -------------------
                g = pool.tile([P, SN, B * 16], F32, tag="g")
                nc.gpsimd.ap_gather(
                    out_ap=g[:].rearrange("p a b -> p (a b)"), in_ap=XT[:],
                    idxs_ap=CT16[:], channels=P, num_elems=IB, d=1,
                    num_idxs=SN * B * 16,
                )
                addr_f = pool.tile([P, SN], F32, tag="addr_f")
                if bf16:
                    g16 = pool.tile([P, SN, B * 16], mybir.dt.bfloat16,
                                    tag="g16")
                    with nc.allow_low_precision(reason="0/1 bits exact"):
                        nc.vector.tensor_copy(out=g16[:], in_=g[:])
                        nc.vector.tensor_tensor(
                            out=g16[:], in0=g16[:],
                            in1=W16[:][:, None, :].to_broadcast(
                                [P, SN, B * 16]),
                            op=ALU.mult)
                    nc.vector.tensor_reduce(out=addr_f[:], in_=g16[:],
                                            axis=AX.X, op=ALU.add)
                else:
                    nc.vector.tensor_tensor(
                        out=g[:], in0=g[:],
                        in1=W[:][:, None, :].to_broadcast([P, SN, B * 16]),
                        op=ALU.mult)
                    with nc.allow_low_precision(reason="sums < 4096, exact"):
                        nc.vector.tensor_reduce(out=addr_f[:], in_=g[:],
                                                axis=AX.X, op=ALU.add)
                ai = pool.tile([P, SN], I32, tag="ai")
                nc.vector.tensor_copy(out=ai[:], in_=addr_f[:])

                # ---- two-level word select (wh 8-way, then wl 16-way) -----
                wh = pool.tile([P, SN], I32, tag="wh")
                nc.vector.tensor_scalar(out=wh[:], in0=ai[:], scalar1=9,
                                        scalar2=None,
                                        op0=ALU.logical_shift_right)
                m1 = pool.tile([P, SN, WH], I32, tag="m1")
                nc.vector.tensor_tensor(
                    out=m1[:], in0=wh[:][:, :, None].to_broadcast([P, SN, WH]),
                    in1=Giota[:][:, None, :].to_broadcast([P, SN, WH]),
                    op=ALU.is_equal)
                nc.vector.tensor_scalar(out=m1[:], in0=m1[:], scalar1=31,
                                        scalar2=None,
                                        op0=ALU.logical_shift_left)
                nc.vector.tensor_scalar(out=m1[:], in0=m1[:], scalar1=31,
                                        scalar2=None,
                                        op0=ALU.arith_shift_right)
                mt = pool.tile([P, SN, WL, WH], I32, tag="mt")
                nc.vector.tensor_tensor(
                    out=mt[:],
                    in0=m1[:][:, :, None, :].to_broadcast([P, SN, WL, WH]),
                    in1=Rt[:], op=ALU.bitwise_and)
                r16 = pool.tile([P, SN, WL], I32, tag="r16")
                nc.vector.tensor_reduce(out=r16[:], in_=mt[:], axis=AX.X,
                                        op=ALU.bitwise_or)
                wl = pool.tile([P, SN], I32, tag="wl")
                nc.vector.tensor_scalar(out=wl[:], in0=ai[:], scalar1=5,
                                        scalar2=15,
                                        op0=ALU.logical_shift_right,
                                        op1=ALU.bitwise_and)
                m2 = pool.tile([P, SN, WL], I32, tag="m2")
                nc.vector.tensor_tensor(
                    out=m2[:], in0=wl[:][:, :, None].to_broadcast([P, SN, WL]),
                    in1=Liota[:][:, None, :].to_broadcast([P, SN, WL]),
                    op=ALU.is_equal)
                nc.vector.tensor_scalar(out=m2[:], in0=m2[:], scalar1=31,
                                        scalar2=None,
                                        op0=ALU.logical_shift_left)
                nc.vector.tensor_scalar(out=m2[:], in0=m2[:], scalar1=31,
                                        scalar2=None,
                                        op0=ALU.arith_shift_right)
                nc.vector.tensor_tensor(out=m2[:], in0=m2[:], in1=r16[:],
                                        op=ALU.bitwise_and)
                wsel = pool.tile([P, SN], I32, tag="wsel")
                nc.vector.tensor_reduce(out=wsel[:], in_=m2[:], axis=AX.X,
                                        op=ALU.bitwise_or)

                # ---- bit extract: (wsel >> (addr&31)) & 1 -----------------
                wlo = pool.tile([P, SN], I32, tag="wlo")
                nc.vector.tensor_scalar(out=wlo[:], in0=ai[:], scalar1=31,
                                        scalar2=None, op0=ALU.bitwise_and)
                nc.vector.tensor_tensor(out=wsel[:], in0=wsel[:], in1=wlo[:],
                                        op=ALU.logical_shift_right)
                nc.vector.tensor_scalar(out=wsel[:], in0=wsel[:], scalar1=1,
                                        scalar2=None, op0=ALU.bitwise_and)
                vals = pool.tile([P, SN], F32, tag="vals")
                nc.vector.tensor_copy(out=vals[:], in_=wsel[:])

                # ---- ensemble vote + threshold ----------------------------
                votes = pool.tile([P, NPP], F32, tag="votes")
                nc.vector.tensor_reduce(
                    out=votes[:],
                    in_=vals[:].rearrange("p (s n1) -> p n1 s", s=S),
                    axis=AX.X, op=ALU.add)
                res = pool.tile([P, NPP], F32, tag="res")
                nc.vector.tensor_scalar(out=res[:], in0=votes[:],
                                        scalar1=float(S) / 2.0, scalar2=None,
                                        op0=ALU.is_gt)
                nc.sync.dma_start(out=y_p, in_=res[:])

            if loop_iters is None:
                body()
            else:
                assert loop_iters % unroll == 0
                with tc.For_i(0, loop_iters // unroll, 1) as _i:
                    for _u in range(unroll):
                        body(_i)

    nc.compile()
    return nc


def _get(loop_iters=None, **flags):
    key = (loop_iters, tuple(sorted(flags.items())))
    if key not in _cache:
        _cache[key] = build(loop_iters, **flags)
    return _cache[key]


def make_in_maps(x, conn, memory):
    """Slice full inputs into per-core input maps (host-side sharding and
    bit-packing of the 0/1 RAM tables only)."""
    mb = np.ascontiguousarray(memory).astype(bool)
    pk = np.packbits(mb, axis=-1, bitorder="little")        # [S, N, A//8] u8
    w32 = pk.view(np.int32)                                 # [S, N, NW]
    # word w = wh*16 + wl stored at [..., wl, wh] for the two-level select
    w32 = np.ascontiguousarray(w32.reshape(S, N, WH, WL).swapaxes(-1, -2))
    ins = []
    for c in range(NCORES):
        lo, hi = c * NPC, (c + 1) * NPC
        wc = (w32[:, lo:hi]
              .reshape(S, P, NPP, NW)
              .transpose(1, 0, 2, 3)
              .reshape(P, SN * NW))
        ins.append({
            "x": np.ascontiguousarray(x).astype(np.int32, copy=False),
            "conn": np.ascontiguousarray(conn[:, lo:hi, :]).astype(
                np.int32, copy=False),
            "mem": np.ascontiguousarray(wc),
        })
    return ins


def kernel(x, conn, memory, *, loop_iters=None):
    # unroll the hardware loop 2x: the For_i iteration boundary costs ~15 us
    # and unrolling also overlaps the table-DMA/sem latency across bodies
    unroll = 2 if loop_iters is not None and loop_iters % 2 == 0 else 1
    nc = _get(loop_iters, unroll=unroll)
    ins = make_in_maps(x, conn, memory)
    res = run_bass_kernel_spmd(nc, ins, core_ids=list(range(NCORES)))
    return np.concatenate([res.results[c]["y"] for c in range(NCORES)]).astype(
        np.float32)
